# revision 16
# baseline (speedup 1.0000x reference)
"""Trainium2 Bass kernel for the AESUELOGIT segment-reduce problem.

Strategy (8 NeuronCores, SPMD):
  - Shard the 20000 paths across cores ALIGNED TO OD BOUNDARIES (core i owns
    ods [500i, 500(i+1)) and all their paths).  The segmented softmax is then
    fully core-local: no denominator collective is needed.
  - Per core: matmul1 vf = V @ D_shard (bf16, V split hi/lo for precision),
    segment sums via small host-built 0/1 segment matrices (PE matmuls over
    chunk-local od slots), gather-back of denominators via matmuls that also
    stitch od segments that straddle 128-path chunk boundaries, matmul2
    x_partial = f @ D_shard^T (bf16), then ONE AllReduce of the (96, 2048)
    partial link flows, and the BPR epilogue fused on-device.
  - D is cast to bf16 on host (its values are exactly 0/1 -> lossless) and
    uploaded in both layouts (link-major for matmul1, path-major for matmul2).
"""

import numpy as np
import ml_dtypes

import concourse.bacc as bacc
import concourse.bass as bass
import concourse.mybir as mybir
import concourse.tile as tile
from concourse.bass_utils import run_bass_kernel_spmd
from concourse.masks import make_identity

F32 = mybir.dt.float32
BF16 = mybir.dt.bfloat16
AF = mybir.ActivationFunctionType
ALU = mybir.AluOpType

ND, NH, NL, NF = 4, 24, 2000, 4
NP, NOD, NCORES = 20000, 4000, 8
DH = ND * NH            # 96
L_PAD = 2048            # links padded to 16*128
KL = L_PAD // 128       # 16 link chunks
EPS = 1e-12


def _build_program(PSHARD, SLOT):
    """Emit the SPMD Bass program (identical on all cores)."""
    NCH = PSHARD // 128
    NPS = (PSHARD + 511) // 512          # psum column chunks for matmul1
    nc = bacc.Bacc("TRN2", target_bir_lowering=False, debug=False,
                   num_devices=NCORES)

    # ---- parameters (per-core shards) ----
    p_ttf = nc.dram_tensor("ttf", [DH, L_PAD], F32, kind="ExternalInput")
    p_ft = nc.dram_tensor("ft", [DH, NF * L_PAD], F32, kind="ExternalInput")
    p_d = nc.dram_tensor("dk", [KL, 128, PSHARD], BF16, kind="ExternalInput")
    p_dt = nc.dram_tensor("dtk", [NCH, 128, L_PAD], BF16, kind="ExternalInput")
    p_s = nc.dram_tensor("seg", [NCH, 128, SLOT], F32, kind="ExternalInput")
    p_sl = nc.dram_tensor("segl", [NCH, 128, 1], F32, kind="ExternalInput")
    p_stm = nc.dram_tensor("stm", [NCH, SLOT, 128], F32, kind="ExternalInput")
    p_stf = nc.dram_tensor("stf", [NCH, 1, 128], F32, kind="ExternalInput")
    p_stb = nc.dram_tensor("stb", [NCH, 1, 128], F32, kind="ExternalInput")
    p_qs = nc.dram_tensor("qsp", [128, NCH], F32, kind="ExternalInput")
    p_th = nc.dram_tensor("th", [1, NF], F32, kind="ExternalInput")
    p_tl = nc.dram_tensor("tl", [128, KL], F32, kind="ExternalInput")
    p_la = nc.dram_tensor("la", [128, KL], F32, kind="ExternalInput")
    p_br = nc.dram_tensor("br", [128, KL], F32, kind="ExternalInput")
    p_kk = nc.dram_tensor("kk", [128, KL], F32, kind="ExternalInput")
    p_out = nc.dram_tensor("out", [KL, 128, DH], F32, kind="ExternalOutput")

    with tile.TileContext(nc) as tc:
        with tc.tile_pool(name="const", bufs=1) as cpool, \
             tc.tile_pool(name="dram", bufs=1, space="DRAM") as dpool, \
             tc.tile_pool(name="big", bufs=1) as bpool, \
             tc.tile_pool(name="stream", bufs=3) as spool:

            ident = cpool.tile([128, 128], F32, tag="ident")
            make_identity(nc, ident[:])

            # small per-link / per-path constants, transposed into SBUF
            tl_sb = cpool.tile([128, KL], F32, tag="tl")
            la_sb = cpool.tile([128, KL], F32, tag="la")
            br_sb = cpool.tile([128, KL], F32, tag="br")
            kk_sb = cpool.tile([128, KL], F32, tag="kk")
            for dst, src in ((tl_sb, p_tl), (la_sb, p_la),
                             (br_sb, p_br), (kk_sb, p_kk)):
                nc.sync.dma_start(dst[:], src.ap())
            qs_sb = cpool.tile([128, NCH], F32, tag="qs")
            nc.sync.dma_start(qs_sb[:], p_qs.ap())
            th_sb = cpool.tile([1, NF], F32, tag="th")
            nc.sync.dma_start(th_sb[:], p_th.ap())

            # q = q_sqrt ** 2
            qsq = cpool.tile([128, NCH], F32, tag="qsq")
            nc.vector.tensor_mul(qsq[:], qs_sb[:], qs_sb[:])
            # alpha = exp(log_alpha); beta = clip(beta_raw, EPS, 4); 1/k
            al_sb = cpool.tile([128, KL], F32, tag="al")
            nc.scalar.activation(al_sb[:], la_sb[:], AF.Exp)
            be_sb = cpool.tile([128, KL], F32, tag="be")
            nc.vector.tensor_scalar(be_sb[:], br_sb[:], float(EPS), 4.0,
                                    ALU.max, ALU.min)
            ik_sb = cpool.tile([128, KL], F32, tag="ik")
            nc.vector.reciprocal(ik_sb[:], kk_sb[:])

            # theta = min(theta_raw, 0), broadcast to all partitions via
            # a K=1 matmul with a ones row.
            thc = cpool.tile([1, NF], F32, tag="thc")
            nc.vector.tensor_scalar_min(thc[:], th_sb[:], 0.0)
            ones = cpool.tile([1, 128], F32, tag="ones")
            nc.vector.memset(ones[:], 1.0)

            # ttf / features
            ttf = bpool.tile([DH, L_PAD], F32, tag="ttf")
            nc.sync.dma_start(ttf[:], p_ttf.ap())
            fts = bpool.tile([DH, NF * L_PAD], F32, tag="fts")
            nc.sync.dma_start(fts[:], p_ft.ap())

            with tc.tile_pool(name="psA", bufs=2, space="PSUM") as psA:
                thb_ps = psA.tile([128, NF], F32, tag="tr")
                nc.tensor.matmul(thb_ps[:], ones[:], thc[:],
                                 start=True, stop=True)
                thb = cpool.tile([128, NF], F32, tag="thb")
                nc.scalar.copy(thb[:], thb_ps[:])

                # V0[dh, l] = sum_f theta_f * feats[f]
                v0 = bpool.tile([DH, L_PAD], F32, tag="v0")
                nc.vector.tensor_scalar_mul(v0[:], fts[:, 0:L_PAD],
                                            thb[:DH, 0:1])
                for f in range(1, NF):
                    nc.vector.scalar_tensor_tensor(
                        v0[:], fts[:, f * L_PAD:(f + 1) * L_PAD],
                        thb[:DH, f:f + 1], v0[:], ALU.mult, ALU.add)

                # ---- matmul1: vf(+theta_links row) = [V;tl]^T-chunks @ D ----
                vf_ps = []
                with tc.tile_pool(name="psV", bufs=1, space="PSUM") as psV:
                    for n in range(NPS):
                        w = min(512, PSHARD - 512 * n)
                        vf_ps.append(psV.tile([97, w], F32, tag=f"vf{n}", name=f"vf{n}"))
                    for k in range(KL):
                        vt_ps = psA.tile([128, DH], F32, tag="tr")
                        nc.tensor.matmul(vt_ps[:], v0[:, 128 * k:128 * (k + 1)],
                                         ident[:DH, :DH], is_transpose=True,
                                         start=True, stop=True)
                        lk = spool.tile([128, 97], F32, tag="lk")
                        nc.scalar.copy(lk[:, 0:DH], vt_ps[:])
                        nc.vector.tensor_copy(lk[:, DH:DH + 1],
                                              tl_sb[:, k:k + 1])
                        lh = spool.tile([128, 97], BF16, tag="lh")
                        nc.vector.tensor_copy(lh[:], lk[:])
                        lup = spool.tile([128, 97], F32, tag="lup")
                        nc.vector.tensor_copy(lup[:], lh[:])
                        nc.vector.tensor_sub(lup[:], lk[:], lup[:])
                        ll = spool.tile([128, 97], BF16, tag="ll")
                        nc.vector.tensor_copy(ll[:], lup[:])

                        dk_t = spool.tile([128, PSHARD], BF16, tag="dk")
                        nc.sync.dma_start(dk_t[:], p_d.ap()[k])
                        for half, lt in ((0, lh), (1, ll)):
                            for n in range(NPS):
                                w = min(512, PSHARD - 512 * n)
                                nc.tensor.matmul(
                                    vf_ps[n][:], lt[:],
                                    dk_t[:, 512 * n:512 * n + w],
                                    start=(k == 0 and half == 0),
                                    stop=(k == KL - 1 and half == 1))

                    # vf psum -> sbuf (still inside psV scope)
                    ysb = bpool.tile([97, PSHARD], F32, tag="ysb")
                    for n in range(NPS):
                        w = min(512, PSHARD - 512 * n)
                        nc.scalar.copy(ysb[:, 512 * n:512 * n + w], vf_ps[n][:])

                # ---- transpose vf chunks, exp -> evT ----
                evt = bpool.tile([128, DH * NCH], F32, tag="evt")
                for c in range(NCH):
                    yt_ps = psA.tile([128, 97], F32, tag="tr")
                    nc.tensor.matmul(yt_ps[:], ysb[:, 128 * c:128 * (c + 1)],
                                     ident[:97, :97], is_transpose=True,
                                     start=True, stop=True)
                    cvec = spool.tile([128, 1], F32, tag="cvec")
                    nc.scalar.copy(cvec[:], yt_ps[:, DH:DH + 1])
                    nc.scalar.activation(evt[:, DH * c:DH * (c + 1)],
                                         yt_ps[:, 0:DH], AF.Exp, bias=cvec[:])

                # ---- segment sums over chunk-local od slots ----
                tall = bpool.tile([SLOT, DH * NCH], F32, tag="tall")
                tlast = bpool.tile([1, DH * NCH], F32, tag="tlast")
                with tc.tile_pool(name="psS", bufs=2, space="PSUM") as psS:
                    for c in range(NCH):
                        s_t = spool.tile([128, SLOT], F32, tag="s")
                        nc.sync.dma_start(s_t[:], p_s.ap()[c])
                        sl_t = spool.tile([128, 1], F32, tag="sl")
                        nc.sync.dma_start(sl_t[:], p_sl.ap()[c])
                        ts_ps = psS.tile([SLOT, DH], F32, tag="seg")
                        nc.tensor.matmul(ts_ps[:], s_t[:],
                                         evt[:, DH * c:DH * (c + 1)],
                                         start=True, stop=True)
                        nc.scalar.copy(tall[:, DH * c:DH * (c + 1)], ts_ps[:])
                        tl_ps = psS.tile([1, DH], F32, tag="segl")
                        nc.tensor.matmul(tl_ps[:], sl_t[:],
                                         evt[:, DH * c:DH * (c + 1)],
                                         start=True, stop=True)
                        nc.scalar.copy(tlast[:, DH * c:DH * (c + 1)], tl_ps[:])

                # ---- gather denominators back per path; f^T = ev*q/denom ----
                ft_bf = bpool.tile([128, DH * NCH], BF16, tag="ftb")
                with tc.tile_pool(name="psG", bufs=2, space="PSUM") as psG:
                    for c in range(NCH):
                        stm_t = spool.tile([SLOT, 128], F32, tag="stm")
                        nc.sync.dma_start(stm_t[:], p_stm.ap()[c])
                        stf_t = spool.tile([1, 128], F32, tag="stf")
                        nc.sync.dma_start(stf_t[:], p_stf.ap()[c])
                        stb_t = spool.tile([1, 128], F32, tag="stb")
                        nc.sync.dma_start(stb_t[:], p_stb.ap()[c])
                        g_ps = psG.tile([128, DH], F32, tag="g")
                        cn = (c + 1) % NCH
                        cp = (c - 1) % NCH
                        nc.tensor.matmul(g_ps[:], stm_t[:],
                                         tall[:, DH * c:DH * (c + 1)],
                                         start=True, stop=False)
                        nc.tensor.matmul(g_ps[:], stf_t[:],
                                         tall[0:1, DH * cn:DH * cn + DH],
                                         start=False, stop=False)
                        nc.tensor.matmul(g_ps[:], stb_t[:],
                                         tlast[0:1, DH * cp:DH * cp + DH],
                                         start=False, stop=True)
                        rec = spool.tile([128, DH], F32, tag="rec")
                        nc.vector.tensor_scalar_max(rec[:], g_ps[:], 1e-30)
                        nc.vector.reciprocal(rec[:], rec[:])
                        tmp = spool.tile([128, DH], F32, tag="tmp")
                        nc.vector.scalar_tensor_tensor(
                            tmp[:], evt[:, DH * c:DH * (c + 1)],
                            qsq[:, c:c + 1], rec[:], ALU.mult, ALU.mult)
                        nc.vector.tensor_copy(ft_bf[:, DH * c:DH * (c + 1)],
                                              tmp[:])

                # ---- matmul2: x_partial = f^T-chunks @ DT ----
                xp = bpool.tile([DH, L_PAD], F32, tag="xp")
                with tc.tile_pool(name="psX", bufs=1, space="PSUM") as psX:
                    x_ps = [psX.tile([DH, 512], F32, tag=f"x{n}", name=f"x{n}")
                            for n in range(L_PAD // 512)]
                    for c in range(NCH):
                        dt_t = spool.tile([128, L_PAD], BF16, tag="dt")
                        nc.sync.dma_start(dt_t[:], p_dt.ap()[c])
                        for n in range(L_PAD // 512):
                            nc.tensor.matmul(
                                x_ps[n][:], ft_bf[:, DH * c:DH * (c + 1)],
                                dt_t[:, 512 * n:512 * (n + 1)],
                                start=(c == 0), stop=(c == NCH - 1))
                    for n in range(L_PAD // 512):
                        nc.scalar.copy(xp[:, 512 * n:512 * (n + 1)], x_ps[n][:])

                # ---- AllReduce partial link flows ----
                ar_in = dpool.tile([DH, L_PAD], F32, tag="arin")
                ar_out = dpool.tile([DH, L_PAD], F32, tag="arout")
                nc.sync.dma_start(ar_in[:], xp[:])
                nc.gpsimd.collective_compute(
                    "AllReduce", ALU.add,
                    replica_groups=[list(range(NCORES))],
                    ins=[ar_in.opt()], outs=[ar_out.opt()])
                xg = bpool.tile([DH, L_PAD], F32, tag="xg")
                nc.sync.dma_start(xg[:], ar_out[:])

                # ---- BPR epilogue in link-major tiles ----
                # (separate psum pool: psA only ever holds the "tr" tag)
                with tc.tile_pool(name="psE", bufs=2, space="PSUM") as psE:
                    for k in range(KL):
                        xt_ps = psE.tile([128, DH], F32, tag="trE")
                        nc.tensor.matmul(xt_ps[:],
                                         xg[:, 128 * k:128 * (k + 1)],
                                         ident[:DH, :DH], is_transpose=True,
                                         start=True, stop=True)
                        tt_ps = psE.tile([128, DH], F32, tag="tr2")
                        nc.tensor.matmul(tt_ps[:],
                                         ttf[:, 128 * k:128 * (k + 1)],
                                         ident[:DH, :DH], is_transpose=True,
                                         start=True, stop=True)
                        xr = spool.tile([128, DH], F32, tag="xr")
                        nc.vector.tensor_scalar_max(xr[:], xt_ps[:], 1e-35)
                        ln_t = spool.tile([128, DH], F32, tag="ln")
                        nc.scalar.activation(ln_t[:], xr[:], AF.Ln,
                                             scale=ik_sb[:, k:k + 1])
                        ex_t = spool.tile([128, DH], F32, tag="ex")
                        nc.scalar.activation(ex_t[:], ln_t[:], AF.Exp,
                                             scale=be_sb[:, k:k + 1])
                        t1 = spool.tile([128, DH], F32, tag="t1")
                        nc.vector.tensor_scalar_mul(t1[:], ex_t[:],
                                                    al_sb[:, k:k + 1])
                        o_t = spool.tile([128, DH], F32, tag="o")
                        nc.vector.scalar_tensor_tensor(o_t[:], t1[:], 1.0,
                                                       tt_ps[:], ALU.add,
                                                       ALU.mult)
                        nc.sync.dma_start(p_out.ap()[k], o_t[:])

    nc.compile()
    return nc


_CACHE = {}
LAST_RESULT = None


def _get_program(PSHARD, SLOT):
    key = (PSHARD, SLOT)
    if key not in _CACHE:
        _CACHE[key] = _build_program(PSHARD, SLOT)
    return _CACHE[key]


def kernel(X, theta_raw, theta_links, q_sqrt, log_alpha, beta_raw, k, D,
           od_of_path, n_ods):
    X = np.asarray(X, np.float32)
    D = np.asarray(D, np.float32)
    od = np.asarray(od_of_path, np.int32)
    assert X.shape == (ND, NH, NL, NF + 1) and D.shape == (NL, NP)
    assert int(n_ods) == NOD

    od_per_core = (NOD + NCORES - 1) // NCORES
    bounds = np.searchsorted(od, np.arange(0, NOD + 1, od_per_core)[:NCORES + 1])
    bounds[0], bounds[-1] = 0, NP
    cnts = np.diff(bounds)
    PSHARD = int(np.ceil(cnts.max() / 128) * 128)
    NCH = PSHARD // 128

    # max od span within any 128-path chunk -> slot count (asc+desc blocks)
    max_span = 1
    for i in range(NCORES):
        odl = od[bounds[i]:bounds[i + 1]]
        for c in range(0, len(odl), 128):
            ch = odl[c:c + 128]
            if len(ch):
                max_span = max(max_span, int(ch[-1] - ch[0]) + 1)
    W = int(np.ceil(max_span / 32) * 32)
    SLOT = W

    nc = _get_program(PSHARD, SLOT)

    # ---- host-side shard construction (index bookkeeping + relayout only) --
    Xf = X.reshape(DH, NL, NF + 1)
    ttf_h = np.zeros((DH, L_PAD), np.float32)
    ttf_h[:, :NL] = Xf[:, :, 0]
    ft_h = np.zeros((DH, NF, L_PAD), np.float32)
    for f in range(NF):
        ft_h[:, f, :NL] = Xf[:, :, f + 1]
    ft_h = np.ascontiguousarray(ft_h.reshape(DH, NF * L_PAD))

    def padded_vec(v, fill=0.0):
        o = np.full(L_PAD, fill, np.float32)
        o[:NL] = v
        # SBUF layout (128 partitions, KL cols): col k holds links
        # [128k, 128(k+1)) -> transpose of the (KL, 128) chunking
        return np.ascontiguousarray(o.reshape(KL, 128).T)

    tl_h = padded_vec(np.asarray(theta_links, np.float32))
    la_h = padded_vec(np.asarray(log_alpha, np.float32))
    br_h = padded_vec(np.asarray(beta_raw, np.float32))
    kk_h = padded_vec(np.asarray(k, np.float32), fill=1.0)
    th_h = np.asarray(theta_raw, np.float32).reshape(1, NF)
    qsr = np.asarray(q_sqrt, np.float32)

    in_maps = []
    for i in range(NCORES):
        lo, hi = bounds[i], bounds[i + 1]
        cnt = hi - lo
        odl = od[lo:hi]

        Dsh = np.zeros((L_PAD, PSHARD), np.float32)
        Dsh[:NL, :cnt] = D[:, lo:hi]
        dk_h = Dsh.astype(ml_dtypes.bfloat16).reshape(KL, 128, PSHARD)
        dt_h = np.ascontiguousarray(Dsh.T).astype(
            ml_dtypes.bfloat16).reshape(NCH, 128, L_PAD)

        s_h = np.zeros((NCH, 128, SLOT), np.float32)
        sl_h = np.zeros((NCH, 128, 1), np.float32)
        stm_h = np.zeros((NCH, SLOT, 128), np.float32)
        stf_h = np.zeros((NCH, 1, 128), np.float32)
        stb_h = np.zeros((NCH, 1, 128), np.float32)
        qs_h = np.zeros(PSHARD, np.float32)
        qs_h[:cnt] = qsr[odl]
        # SBUF layout (128, NCH): col c = paths [128c, 128(c+1))
        qs_h = np.ascontiguousarray(qs_h.reshape(NCH, 128).T)

        firsts, lasts = {}, {}
        for c in range(NCH):
            ch = odl[128 * c:128 * (c + 1)]
            if len(ch):
                firsts[c], lasts[c] = int(ch[0]), int(ch[-1])
        for c in range(NCH):
            ch = odl[128 * c:128 * (c + 1)]
            if not len(ch):
                continue
            f0, l0 = firsts[c], lasts[c]
            asc = ch - f0                      # ascending slots [0, W)
            rows = np.arange(len(ch))
            s_h[c, rows, asc] = 1.0
            sl_h[c, rows[ch == l0], 0] = 1.0   # last-od partial (for stb)
            stm_h[c, asc, rows] = 1.0
            if c + 1 in firsts and firsts[c + 1] == l0:
                stf_h[c, 0, rows[ch == l0]] = 1.0
            if c - 1 in lasts and lasts[c - 1] == f0:
                stb_h[c, 0, rows[ch == f0]] = 1.0

        in_maps.append(dict(
            ttf=ttf_h, ft=ft_h, dk=dk_h, dtk=dt_h, seg=s_h, segl=sl_h, stm=stm_h,
            stf=stf_h, stb=stb_h, qsp=qs_h, th=th_h, tl=tl_h, la=la_h,
            br=br_h, kk=kk_h))

    import os
    trace = os.environ.get("BASS_KERNEL_TRACE", "0") == "1"
    res = run_bass_kernel_spmd(nc, in_maps, core_ids=list(range(NCORES)),
                               trace=trace)
    global LAST_RESULT
    LAST_RESULT = res
    out_t = res.results[0]["out"]              # (KL, 128, DH)
    out = out_t.reshape(L_PAD, DH)[:NL].T      # (DH, NL)
    return np.ascontiguousarray(out).reshape(ND, NH, NL).astype(np.float32)


# revision 17
# speedup vs baseline: 1.2657x; 1.2657x over previous
"""Trainium2 Bass kernel for the AESUELOGIT segment-reduce problem.

Strategy (8 NeuronCores, SPMD):
  - Shard the 20000 paths across cores ALIGNED TO OD BOUNDARIES (core i owns
    ods [500i, 500(i+1)) and all their paths).  The segmented softmax is then
    fully core-local: no denominator collective is needed.
  - Per core: matmul1 vf = V @ D_shard (bf16, V split hi/lo for precision),
    segment sums via small host-built 0/1 segment matrices (bf16 PE matmuls
    over chunk-local od slots), gather-back of denominators via matmuls that
    also stitch od segments straddling 128-path chunk boundaries, matmul2
    x_partial = f @ D_shard^T (bf16), a ReduceScatter of the (96, 2048)
    partial link flows (each core epilogues its own 12 day-hour rows), and
    the BPR epilogue fused on-device; host concatenates the 8 slices.
  - D is cast to bf16 on host (its values are exactly 0/1 -> lossless) and
    uploaded in both layouts (link-major for matmul1, path-major for matmul2).
"""

import os

import numpy as np
import ml_dtypes

import concourse.bacc as bacc
import concourse.bass as bass
import concourse.mybir as mybir
import concourse.tile as tile
from concourse.bass_utils import run_bass_kernel_spmd
from concourse.masks import make_identity

F32 = mybir.dt.float32
BF16 = mybir.dt.bfloat16
AF = mybir.ActivationFunctionType
ALU = mybir.AluOpType

ND, NH, NL, NF = 4, 24, 2000, 4
NP, NOD, NCORES = 20000, 4000, 8
DH = ND * NH            # 96
DHS = DH // NCORES      # 12 day-hour rows per core after ReduceScatter
L_PAD = 2048            # links padded to 16*128
KL = L_PAD // 128       # 16 link chunks
EPS = 1e-12


def _build_program(PSHARD, SLOT):
    """Emit the SPMD Bass program (identical on all cores)."""
    NCH = PSHARD // 128
    NPS = (PSHARD + 511) // 512          # psum column chunks for matmul1
    nc = bacc.Bacc("TRN2", target_bir_lowering=False, debug=False,
                   num_devices=NCORES)

    # ---- parameters (per-core shards) ----
    p_ttf = nc.dram_tensor("ttfs", [DHS, L_PAD], F32, kind="ExternalInput")
    p_ft = nc.dram_tensor("ft", [DH, NF * L_PAD], F32, kind="ExternalInput")
    p_d = nc.dram_tensor("dk", [KL, 128, PSHARD], BF16, kind="ExternalInput")
    p_dt = nc.dram_tensor("dtk", [NCH, 128, L_PAD], BF16, kind="ExternalInput")
    p_s = nc.dram_tensor("seg", [NCH, 128, SLOT], BF16, kind="ExternalInput")
    p_sl = nc.dram_tensor("segl", [NCH, 128, 1], BF16, kind="ExternalInput")
    p_stm = nc.dram_tensor("stm", [NCH, SLOT, 128], BF16, kind="ExternalInput")
    p_stf = nc.dram_tensor("stf", [NCH, 1, 128], BF16, kind="ExternalInput")
    p_stb = nc.dram_tensor("stb", [NCH, 1, 128], BF16, kind="ExternalInput")
    p_qs = nc.dram_tensor("qsp", [128, NCH], F32, kind="ExternalInput")
    p_th = nc.dram_tensor("th", [1, NF], F32, kind="ExternalInput")
    p_tl = nc.dram_tensor("tl", [128, KL], F32, kind="ExternalInput")
    p_la = nc.dram_tensor("la", [128, KL], F32, kind="ExternalInput")
    p_br = nc.dram_tensor("br", [128, KL], F32, kind="ExternalInput")
    p_kk = nc.dram_tensor("kk", [128, KL], F32, kind="ExternalInput")
    p_out = nc.dram_tensor("out", [KL, 128, DHS], F32, kind="ExternalOutput")

    with tile.TileContext(nc) as tc:
        with tc.tile_pool(name="const", bufs=1) as cpool, \
             tc.tile_pool(name="dram", bufs=1, space="DRAM") as dpool, \
             tc.tile_pool(name="big", bufs=1) as bpool, \
             tc.tile_pool(name="stream", bufs=3) as spool:

            ident = cpool.tile([128, 128], F32, tag="ident")
            make_identity(nc, ident[:])

            tl_sb = cpool.tile([128, KL], F32, tag="tl")
            la_sb = cpool.tile([128, KL], F32, tag="la")
            br_sb = cpool.tile([128, KL], F32, tag="br")
            kk_sb = cpool.tile([128, KL], F32, tag="kk")
            for dst, src in ((tl_sb, p_tl), (la_sb, p_la),
                             (br_sb, p_br), (kk_sb, p_kk)):
                nc.sync.dma_start(dst[:], src.ap())
            qs_sb = cpool.tile([128, NCH], F32, tag="qs")
            nc.sync.dma_start(qs_sb[:], p_qs.ap())
            th_sb = cpool.tile([1, NF], F32, tag="th")
            nc.sync.dma_start(th_sb[:], p_th.ap())

            qsq = cpool.tile([128, NCH], F32, tag="qsq")
            nc.vector.tensor_mul(qsq[:], qs_sb[:], qs_sb[:])
            al_sb = cpool.tile([128, KL], F32, tag="al")
            nc.scalar.activation(al_sb[:], la_sb[:], AF.Exp)
            be_sb = cpool.tile([128, KL], F32, tag="be")
            nc.vector.tensor_scalar(be_sb[:], br_sb[:], float(EPS), 4.0,
                                    ALU.max, ALU.min)
            ik_sb = cpool.tile([128, KL], F32, tag="ik")
            nc.vector.reciprocal(ik_sb[:], kk_sb[:])

            thc = cpool.tile([1, NF], F32, tag="thc")
            nc.vector.tensor_scalar_min(thc[:], th_sb[:], 0.0)
            ones = cpool.tile([1, 128], F32, tag="ones")
            nc.vector.memset(ones[:], 1.0)

            ttfs = cpool.tile([DHS, L_PAD], F32, tag="ttfs")
            nc.sync.dma_start(ttfs[:], p_ttf.ap())
            fts = bpool.tile([DH, NF * L_PAD], F32, tag="fts")
            nc.sync.dma_start(fts[:], p_ft.ap())

            with tc.tile_pool(name="psA", bufs=2, space="PSUM") as psA:
                thb_ps = psA.tile([128, NF], F32, tag="tr")
                nc.tensor.matmul(thb_ps[:], ones[:], thc[:],
                                 start=True, stop=True)
                thb = cpool.tile([128, NF], F32, tag="thb")
                nc.scalar.copy(thb[:], thb_ps[:])

                # V0[dh, l] = sum_f theta_f * feats[f]
                v0 = bpool.tile([DH, L_PAD], F32, tag="v0")
                nc.vector.tensor_scalar_mul(v0[:], fts[:, 0:L_PAD],
                                            thb[:DH, 0:1])
                for f in range(1, NF):
                    nc.vector.scalar_tensor_tensor(
                        v0[:], fts[:, f * L_PAD:(f + 1) * L_PAD],
                        thb[:DH, f:f + 1], v0[:], ALU.mult, ALU.add)

                # ---- matmul1: vf(+theta_links row) = [V;tl]^T-chunks @ D ----
                # lhsT padded to 128 cols so the fast-weight-load path kicks in
                vf_ps = []
                with tc.tile_pool(name="psV", bufs=1, space="PSUM") as psV:
                    for n in range(NPS):
                        w = min(512, PSHARD - 512 * n)
                        vf_ps.append(psV.tile([128, w], F32, tag=f"vf{n}",
                                              name=f"vf{n}"))
                    for k in range(KL):
                        vt_ps = psA.tile([128, DH], F32, tag="tr")
                        nc.tensor.matmul(vt_ps[:], v0[:, 128 * k:128 * (k + 1)],
                                         ident[:DH, :DH], is_transpose=True,
                                         start=True, stop=True)
                        lk = spool.tile([128, 128], F32, tag="lk")
                        nc.scalar.copy(lk[:, 0:DH], vt_ps[:])
                        nc.vector.tensor_copy(lk[:, DH:DH + 1],
                                              tl_sb[:, k:k + 1])
                        nc.vector.memset(lk[:, DH + 1:128], 0.0)
                        lh = spool.tile([128, 128], BF16, tag="lh")
                        nc.vector.tensor_copy(lh[:], lk[:])
                        lup = spool.tile([128, 128], F32, tag="lup")
                        nc.vector.tensor_copy(lup[:], lh[:])
                        nc.vector.tensor_sub(lup[:], lk[:], lup[:])
                        ll = spool.tile([128, 128], BF16, tag="ll")
                        nc.vector.tensor_copy(ll[:], lup[:])

                        dk_t = spool.tile([128, PSHARD], BF16, tag="dk")
                        nc.sync.dma_start(dk_t[:], p_d.ap()[k])
                        for half, lt in ((0, lh), (1, ll)):
                            for n in range(NPS):
                                w = min(512, PSHARD - 512 * n)
                                nc.tensor.matmul(
                                    vf_ps[n][:], lt[:],
                                    dk_t[:, 512 * n:512 * n + w],
                                    start=(k == 0 and half == 0),
                                    stop=(k == KL - 1 and half == 1))

                    ysb = bpool.tile([97, PSHARD], F32, tag="ysb")
                    for n in range(NPS):
                        w = min(512, PSHARD - 512 * n)
                        nc.scalar.copy(ysb[:, 512 * n:512 * n + w],
                                       vf_ps[n][0:97, :])

                # ---- transpose vf chunks, exp -> evT (f32 + bf16 copies) ----
                evt = bpool.tile([128, DH * NCH], F32, tag="evt")
                evb = bpool.tile([128, DH * NCH], BF16, tag="evb")
                for c in range(NCH):
                    yt_ps = psA.tile([128, 97], F32, tag="tr")
                    nc.tensor.matmul(yt_ps[:], ysb[:, 128 * c:128 * (c + 1)],
                                     ident[:97, :97], is_transpose=True,
                                     start=True, stop=True)
                    cvec = spool.tile([128, 1], F32, tag="cvec")
                    nc.scalar.copy(cvec[:], yt_ps[:, DH:DH + 1])
                    nc.scalar.activation(evt[:, DH * c:DH * (c + 1)],
                                         yt_ps[:, 0:DH], AF.Exp, bias=cvec[:])
                    nc.vector.tensor_copy(evb[:, DH * c:DH * (c + 1)],
                                          evt[:, DH * c:DH * (c + 1)])

                # ---- segment sums over chunk-local od slots (bf16) ----
                tall = bpool.tile([SLOT, DH * NCH], BF16, tag="tall")
                tlast = bpool.tile([1, DH * NCH], BF16, tag="tlast")
                with tc.tile_pool(name="psS", bufs=2, space="PSUM") as psS:
                    for c in range(NCH):
                        s_t = spool.tile([128, SLOT], BF16, tag="s")
                        nc.sync.dma_start(s_t[:], p_s.ap()[c])
                        sl_t = spool.tile([128, 1], BF16, tag="sl")
                        nc.sync.dma_start(sl_t[:], p_sl.ap()[c])
                        ts_ps = psS.tile([SLOT, DH], F32, tag="seg")
                        nc.tensor.matmul(ts_ps[:], s_t[:],
                                         evb[:, DH * c:DH * (c + 1)],
                                         start=True, stop=True)
                        nc.scalar.copy(tall[:, DH * c:DH * (c + 1)], ts_ps[:])
                        tl_ps = psS.tile([1, DH], F32, tag="segl")
                        nc.tensor.matmul(tl_ps[:], sl_t[:],
                                         evb[:, DH * c:DH * (c + 1)],
                                         start=True, stop=True)
                        nc.scalar.copy(tlast[:, DH * c:DH * (c + 1)], tl_ps[:])

                # ---- gather denominators per path; f^T = ev*q/denom ----
                ft_bf = bpool.tile([128, 128 * NCH], BF16, tag="ftb")
                nc.vector.memset(ft_bf[:], 0.0)
                with tc.tile_pool(name="psG", bufs=2, space="PSUM") as psG:
                    for c in range(NCH):
                        stm_t = spool.tile([SLOT, 128], BF16, tag="stm")
                        nc.sync.dma_start(stm_t[:], p_stm.ap()[c])
                        stf_t = spool.tile([1, 128], BF16, tag="stf")
                        nc.sync.dma_start(stf_t[:], p_stf.ap()[c])
                        stb_t = spool.tile([1, 128], BF16, tag="stb")
                        nc.sync.dma_start(stb_t[:], p_stb.ap()[c])
                        g_ps = psG.tile([128, DH], F32, tag="g")
                        cn = (c + 1) % NCH
                        cp = (c - 1) % NCH
                        nc.tensor.matmul(g_ps[:], stm_t[:],
                                         tall[:, DH * c:DH * (c + 1)],
                                         start=True, stop=False)
                        nc.tensor.matmul(g_ps[:], stf_t[:],
                                         tall[0:1, DH * cn:DH * cn + DH],
                                         start=False, stop=False)
                        nc.tensor.matmul(g_ps[:], stb_t[:],
                                         tlast[0:1, DH * cp:DH * cp + DH],
                                         start=False, stop=True)
                        rec = spool.tile([128, DH], F32, tag="rec")
                        nc.vector.tensor_scalar_max(rec[:], g_ps[:], 1e-30)
                        nc.vector.reciprocal(rec[:], rec[:])
                        tmp = spool.tile([128, DH], F32, tag="tmp")
                        nc.vector.scalar_tensor_tensor(
                            tmp[:], evt[:, DH * c:DH * (c + 1)],
                            qsq[:, c:c + 1], rec[:], ALU.mult, ALU.mult)
                        nc.vector.tensor_copy(
                            ft_bf[:, 128 * c:128 * c + DH], tmp[:])

                # ---- matmul2: x_partial = f^T-chunks @ DT ----
                xp = bpool.tile([DH, L_PAD], F32, tag="xp")
                with tc.tile_pool(name="psX", bufs=1, space="PSUM") as psX:
                    x_ps = [psX.tile([128, 512], F32, tag=f"x{n}", name=f"x{n}")
                            for n in range(L_PAD // 512)]
                    for c in range(NCH):
                        dt_t = spool.tile([128, L_PAD], BF16, tag="dt")
                        nc.sync.dma_start(dt_t[:], p_dt.ap()[c])
                        for n in range(L_PAD // 512):
                            nc.tensor.matmul(
                                x_ps[n][:], ft_bf[:, 128 * c:128 * (c + 1)],
                                dt_t[:, 512 * n:512 * (n + 1)],
                                start=(c == 0), stop=(c == NCH - 1))
                    for n in range(L_PAD // 512):
                        nc.scalar.copy(xp[:, 512 * n:512 * (n + 1)],
                                       x_ps[n][0:DH, :])

                # ---- ReduceScatter partial link flows; each core keeps its
                # own DHS day-hour rows ----
                ar_in = dpool.tile([DH, L_PAD], F32, tag="arin")
                ar_out = dpool.tile([DHS, L_PAD], F32, tag="arout")
                nc.sync.dma_start(ar_in[:], xp[:])
                nc.gpsimd.collective_compute(
                    "ReduceScatter", ALU.add,
                    replica_groups=[list(range(NCORES))],
                    ins=[ar_in.opt()], outs=[ar_out.opt()])
                xg = bpool.tile([DHS, L_PAD], F32, tag="xg")
                nc.sync.dma_start(xg[:], ar_out[:])

                # ---- BPR epilogue on the local (DHS, L_PAD) slice.
                # Transposes first, then batched Ln then batched Exp so the
                # activation tables load only once each. ----
                xrT = bpool.tile([128, DHS * KL], F32, tag="xrT")
                ttT = bpool.tile([128, DHS * KL], F32, tag="ttT")
                lnT = bpool.tile([128, DHS * KL], F32, tag="lnT")
                exT = bpool.tile([128, DHS * KL], F32, tag="exT")
                with tc.tile_pool(name="psE", bufs=2, space="PSUM") as psE:
                    for k in range(KL):
                        xt_ps = psE.tile([128, DHS], F32, tag="trE")
                        nc.tensor.matmul(xt_ps[:],
                                         xg[:, 128 * k:128 * (k + 1)],
                                         ident[:DHS, :DHS], is_transpose=True,
                                         start=True, stop=True)
                        nc.vector.tensor_scalar_max(
                            xrT[:, DHS * k:DHS * (k + 1)], xt_ps[:], 1e-35)
                        tt_ps = psE.tile([128, DHS], F32, tag="tr2")
                        nc.tensor.matmul(tt_ps[:],
                                         ttfs[:, 128 * k:128 * (k + 1)],
                                         ident[:DHS, :DHS], is_transpose=True,
                                         start=True, stop=True)
                        nc.scalar.copy(ttT[:, DHS * k:DHS * (k + 1)], tt_ps[:])
                for k in range(KL):
                    nc.scalar.activation(lnT[:, DHS * k:DHS * (k + 1)],
                                         xrT[:, DHS * k:DHS * (k + 1)],
                                         AF.Ln, scale=ik_sb[:, k:k + 1])
                for k in range(KL):
                    nc.scalar.activation(exT[:, DHS * k:DHS * (k + 1)],
                                         lnT[:, DHS * k:DHS * (k + 1)],
                                         AF.Exp, scale=be_sb[:, k:k + 1])
                for k in range(KL):
                    t1 = spool.tile([128, DHS], F32, tag="t1")
                    nc.vector.tensor_scalar_mul(t1[:],
                                                exT[:, DHS * k:DHS * (k + 1)],
                                                al_sb[:, k:k + 1])
                    o_t = spool.tile([128, DHS], F32, tag="o")
                    nc.vector.scalar_tensor_tensor(
                        o_t[:], t1[:], 1.0, ttT[:, DHS * k:DHS * (k + 1)],
                        ALU.add, ALU.mult)
                    nc.sync.dma_start(p_out.ap()[k], o_t[:])

    nc.compile()
    return nc


_CACHE = {}
LAST_RESULT = None


def _get_program(PSHARD, SLOT):
    key = (PSHARD, SLOT)
    if key not in _CACHE:
        _CACHE[key] = _build_program(PSHARD, SLOT)
    return _CACHE[key]


def kernel(X, theta_raw, theta_links, q_sqrt, log_alpha, beta_raw, k, D,
           od_of_path, n_ods):
    X = np.asarray(X, np.float32)
    D = np.asarray(D, np.float32)
    od = np.asarray(od_of_path, np.int32)
    assert X.shape == (ND, NH, NL, NF + 1) and D.shape == (NL, NP)
    assert int(n_ods) == NOD

    od_per_core = (NOD + NCORES - 1) // NCORES
    bounds = np.searchsorted(od, np.arange(0, NOD + 1, od_per_core)[:NCORES + 1])
    bounds[0], bounds[-1] = 0, NP
    cnts = np.diff(bounds)
    PSHARD = int(np.ceil(cnts.max() / 128) * 128)
    NCH = PSHARD // 128

    max_span = 1
    for i in range(NCORES):
        odl = od[bounds[i]:bounds[i + 1]]
        for c in range(0, len(odl), 128):
            ch = odl[c:c + 128]
            if len(ch):
                max_span = max(max_span, int(ch[-1] - ch[0]) + 1)
    W = int(np.ceil(max_span / 32) * 32)
    SLOT = W

    nc = _get_program(PSHARD, SLOT)

    # ---- host-side shard construction (index bookkeeping + relayout only) --
    Xf = X.reshape(DH, NL, NF + 1)
    ttf_full = np.zeros((DH, L_PAD), np.float32)
    ttf_full[:, :NL] = Xf[:, :, 0]
    ft_h = np.zeros((DH, NF, L_PAD), np.float32)
    for f in range(NF):
        ft_h[:, f, :NL] = Xf[:, :, f + 1]
    ft_h = np.ascontiguousarray(ft_h.reshape(DH, NF * L_PAD))

    def padded_vec(v, fill=0.0):
        o = np.full(L_PAD, fill, np.float32)
        o[:NL] = v
        return np.ascontiguousarray(o.reshape(KL, 128).T)

    tl_h = padded_vec(np.asarray(theta_links, np.float32))
    la_h = padded_vec(np.asarray(log_alpha, np.float32))
    br_h = padded_vec(np.asarray(beta_raw, np.float32))
    kk_h = padded_vec(np.asarray(k, np.float32), fill=1.0)
    th_h = np.asarray(theta_raw, np.float32).reshape(1, NF)
    qsr = np.asarray(q_sqrt, np.float32)

    in_maps = []
    for i in range(NCORES):
        lo, hi = bounds[i], bounds[i + 1]
        cnt = hi - lo
        odl = od[lo:hi]

        Dsh = np.zeros((L_PAD, PSHARD), np.float32)
        Dsh[:NL, :cnt] = D[:, lo:hi]
        dk_h = Dsh.astype(ml_dtypes.bfloat16).reshape(KL, 128, PSHARD)
        dt_h = np.ascontiguousarray(Dsh.T).astype(
            ml_dtypes.bfloat16).reshape(NCH, 128, L_PAD)

        s_h = np.zeros((NCH, 128, SLOT), ml_dtypes.bfloat16)
        sl_h = np.zeros((NCH, 128, 1), ml_dtypes.bfloat16)
        stm_h = np.zeros((NCH, SLOT, 128), ml_dtypes.bfloat16)
        stf_h = np.zeros((NCH, 1, 128), ml_dtypes.bfloat16)
        stb_h = np.zeros((NCH, 1, 128), ml_dtypes.bfloat16)
        qs_h = np.zeros(PSHARD, np.float32)
        qs_h[:cnt] = qsr[odl]
        qs_h = np.ascontiguousarray(qs_h.reshape(NCH, 128).T)

        firsts, lasts = {}, {}
        for c in range(NCH):
            ch = odl[128 * c:128 * (c + 1)]
            if len(ch):
                firsts[c], lasts[c] = int(ch[0]), int(ch[-1])
        for c in range(NCH):
            ch = odl[128 * c:128 * (c + 1)]
            if not len(ch):
                continue
            f0, l0 = firsts[c], lasts[c]
            asc = ch - f0
            rows = np.arange(len(ch))
            s_h[c, rows, asc] = 1.0
            sl_h[c, rows[ch == l0], 0] = 1.0
            stm_h[c, asc, rows] = 1.0
            if c + 1 in firsts and firsts[c + 1] == l0:
                stf_h[c, 0, rows[ch == l0]] = 1.0
            if c - 1 in lasts and lasts[c - 1] == f0:
                stb_h[c, 0, rows[ch == f0]] = 1.0

        in_maps.append(dict(
            ttfs=np.ascontiguousarray(ttf_full[DHS * i:DHS * (i + 1)]),
            ft=ft_h, dk=dk_h, dtk=dt_h, seg=s_h, segl=sl_h, stm=stm_h,
            stf=stf_h, stb=stb_h, qsp=qs_h, th=th_h, tl=tl_h, la=la_h,
            br=br_h, kk=kk_h))

    trace = os.environ.get("BASS_KERNEL_TRACE", "0") == "1"
    res = run_bass_kernel_spmd(nc, in_maps, core_ids=list(range(NCORES)),
                               trace=trace)
    global LAST_RESULT
    LAST_RESULT = res
    # core i's output holds day-hour rows [DHS*i, DHS*(i+1)) in link-major
    # (KL, 128, DHS) tiles -> concatenate and transpose back
    out_t = np.concatenate([r["out"] for r in res.results], axis=2)
    out = out_t.reshape(L_PAD, DH)[:NL].T
    return np.ascontiguousarray(out).reshape(ND, NH, NL).astype(np.float32)


# revision 20
# speedup vs baseline: 1.7949x; 1.4182x over previous
"""Trainium2 Bass kernel for the AESUELOGIT segment-reduce problem.

Strategy (8 NeuronCores, SPMD):
  - Shard the 20000 paths across cores ALIGNED TO OD BOUNDARIES (core i owns
    ods [500i, 500(i+1)) and all their paths).  The segmented softmax is then
    fully core-local: no denominator collective is needed.
  - Per core: matmul1 vf = V @ D_shard (bf16, V split hi/lo for precision),
    segment sums via small host-built 0/1 segment matrices (bf16 PE matmuls
    over chunk-local od slots), gather-back of denominators via matmuls that
    also stitch od segments straddling 128-path chunk boundaries, matmul2
    x_partial = f @ D_shard^T (bf16), a ReduceScatter of the (96, 2048)
    partial link flows (each core epilogues its own 12 day-hour rows), and
    the BPR epilogue fused on-device; host concatenates the 8 slices.
  - D is cast to bf16 on host (its values are exactly 0/1 -> lossless) and
    uploaded in both layouts (link-major for matmul1, path-major for matmul2).
  - The epilogue runs in a folded (96, 256) layout (12 day-hour rows x 8 link
    blocks stacked on partitions) so it needs no transposes and only ~8 ops.
"""

import os

import numpy as np
import ml_dtypes

import concourse.bacc as bacc
import concourse.bass as bass
import concourse.mybir as mybir
import concourse.tile as tile
from concourse.bass_utils import run_bass_kernel_spmd

F32 = mybir.dt.float32
BF16 = mybir.dt.bfloat16
AF = mybir.ActivationFunctionType
ALU = mybir.AluOpType

ND, NH, NL, NF = 4, 24, 2000, 4
NP, NOD, NCORES = 20000, 4000, 8
DH = ND * NH            # 96
DHS = DH // NCORES      # 12 day-hour rows per core after ReduceScatter
L_PAD = 2048            # links padded to 16*128
KL = L_PAD // 128       # 16 link chunks
FB = L_PAD // 256       # 8 link blocks in the folded epilogue layout
EPS = 1e-12


def _build_program(PSHARD, SLOT):
    """Emit the SPMD Bass program (identical on all cores)."""
    NCH = PSHARD // 128
    NPS = (PSHARD + 511) // 512          # psum column chunks for matmul1
    nc = bacc.Bacc("TRN2", target_bir_lowering=False, debug=False,
                   num_devices=NCORES)

    # ---- parameters (per-core shards) ----
    p_ft = nc.dram_tensor("ft", [DH, NF * L_PAD], F32, kind="ExternalInput")
    p_d = nc.dram_tensor("dk", [KL, 128, PSHARD], BF16, kind="ExternalInput")
    p_dt = nc.dram_tensor("dtk", [NCH, 128, L_PAD], BF16, kind="ExternalInput")
    # segment matrices, pre-laid-out for single bulk DMAs
    p_s = nc.dram_tensor("seg", [128, NCH * SLOT], BF16, kind="ExternalInput")
    p_sl = nc.dram_tensor("segl", [128, NCH], BF16, kind="ExternalInput")
    p_stm = nc.dram_tensor("stm", [SLOT, NCH * 128], BF16,
                           kind="ExternalInput")
    p_stf = nc.dram_tensor("stf", [1, NCH * 128], BF16, kind="ExternalInput")
    p_stb = nc.dram_tensor("stb", [1, NCH * 128], BF16, kind="ExternalInput")
    p_qs = nc.dram_tensor("qsp", [128, NCH], F32, kind="ExternalInput")
    p_th = nc.dram_tensor("th", [1, NF], F32, kind="ExternalInput")
    p_tl = nc.dram_tensor("tl", [128, KL], F32, kind="ExternalInput")
    p_id = nc.dram_tensor("idn", [128, 128], F32, kind="ExternalInput")
    # folded (96, 256) epilogue tensors (host-replicated layouts, no math)
    p_kb = nc.dram_tensor("kb96", [DH, 256], F32, kind="ExternalInput")
    p_bb = nc.dram_tensor("bb96", [DH, 256], F32, kind="ExternalInput")
    p_lab = nc.dram_tensor("lab96", [DH, 256], F32, kind="ExternalInput")
    p_ttf = nc.dram_tensor("ttf96", [DH, 256], F32, kind="ExternalInput")
    p_out = nc.dram_tensor("out", [DH, 256], F32, kind="ExternalOutput")

    with tile.TileContext(nc) as tc:
        with tc.tile_pool(name="const", bufs=1) as cpool, \
             tc.tile_pool(name="dram", bufs=1, space="DRAM") as dpool, \
             tc.tile_pool(name="big", bufs=1) as bpool, \
             tc.tile_pool(name="stream", bufs=4) as spool, \
             tc.tile_pool(name="dtp", bufs=6) as dtpool:

            ident = cpool.tile([128, 128], F32, tag="ident")
            nc.sync.dma_start(ident[:], p_id.ap())

            tl_sb = cpool.tile([128, KL], F32, tag="tl")
            nc.sync.dma_start(tl_sb[:], p_tl.ap())
            qs_sb = cpool.tile([128, NCH], F32, tag="qs")
            nc.sync.dma_start(qs_sb[:], p_qs.ap())
            th_sb = cpool.tile([1, NF], F32, tag="th")
            nc.sync.dma_start(th_sb[:], p_th.ap())

            # bulk segment-matrix loads (persistent in SBUF)
            s_all = cpool.tile([128, NCH * SLOT], BF16, tag="s_all")
            nc.sync.dma_start(s_all[:], p_s.ap())
            sl_all = cpool.tile([128, NCH], BF16, tag="sl_all")
            nc.sync.dma_start(sl_all[:], p_sl.ap())
            stm_all = cpool.tile([SLOT, NCH * 128], BF16, tag="stm_all")
            nc.sync.dma_start(stm_all[:], p_stm.ap())
            stf_all = cpool.tile([1, NCH * 128], BF16, tag="stf_all")
            nc.sync.dma_start(stf_all[:], p_stf.ap())
            stb_all = cpool.tile([1, NCH * 128], BF16, tag="stb_all")
            nc.sync.dma_start(stb_all[:], p_stb.ap())

            qsq = cpool.tile([128, NCH], F32, tag="qsq")
            nc.vector.tensor_mul(qsq[:], qs_sb[:], qs_sb[:])

            # folded epilogue constants (prepped early, used after the RS)
            kb = cpool.tile([DH, 256], F32, tag="kb")
            nc.sync.dma_start(kb[:], p_kb.ap())
            bb = cpool.tile([DH, 256], F32, tag="bb")
            nc.sync.dma_start(bb[:], p_bb.ap())
            lab = cpool.tile([DH, 256], F32, tag="lab")
            nc.sync.dma_start(lab[:], p_lab.ap())
            ttf = cpool.tile([DH, 256], F32, tag="ttf")
            nc.sync.dma_start(ttf[:], p_ttf.ap())
            ib = cpool.tile([DH, 256], F32, tag="ib")
            nc.vector.reciprocal(ib[:], kb[:])
            bb2 = cpool.tile([DH, 256], F32, tag="bb2")
            nc.vector.tensor_scalar(bb2[:], bb[:], float(EPS), 4.0,
                                    ALU.max, ALU.min)
            ab = cpool.tile([DH, 256], F32, tag="ab")
            nc.scalar.activation(ab[:], lab[:], AF.Exp)
            atf = cpool.tile([DH, 256], F32, tag="atf")
            nc.vector.tensor_mul(atf[:], ab[:], ttf[:])

            thc = cpool.tile([1, NF], F32, tag="thc")
            nc.vector.tensor_scalar_min(thc[:], th_sb[:], 0.0)
            ones = cpool.tile([1, 128], F32, tag="ones")
            nc.vector.memset(ones[:], 1.0)

            # features, one DMA per feature so V0 accumulation can pipeline
            fts = bpool.tile([DH, NF * L_PAD], F32, tag="fts")
            for f in range(NF):
                nc.sync.dma_start(fts[:, f * L_PAD:(f + 1) * L_PAD],
                                  p_ft.ap()[:, f * L_PAD:(f + 1) * L_PAD])

            with tc.tile_pool(name="psA", bufs=2, space="PSUM") as psA:
                thb_ps = psA.tile([128, NF], F32, tag="tr")
                nc.tensor.matmul(thb_ps[:], ones[:], thc[:],
                                 start=True, stop=True)
                thb = cpool.tile([128, NF], F32, tag="thb")
                nc.scalar.copy(thb[:], thb_ps[:])

                # V0[dh, l] = sum_f theta_f * feats[f]
                v0 = bpool.tile([DH, L_PAD], F32, tag="v0")
                nc.vector.tensor_scalar_mul(v0[:], fts[:, 0:L_PAD],
                                            thb[:DH, 0:1])
                for f in range(1, NF):
                    nc.vector.scalar_tensor_tensor(
                        v0[:], fts[:, f * L_PAD:(f + 1) * L_PAD],
                        thb[:DH, f:f + 1], v0[:], ALU.mult, ALU.add)

                # ---- lhsT prep for matmul1, all chunks upfront so the
                # matmul stream below is dense bf16 ----
                lk_all = bpool.tile([128, 128 * KL], F32, tag="lk_all")
                nc.vector.memset(lk_all[:], 0.0)
                for k in range(KL):
                    vt_ps = psA.tile([128, DH], F32, tag="tr")
                    nc.tensor.matmul(vt_ps[:], v0[:, 128 * k:128 * (k + 1)],
                                     ident[:DH, :DH], is_transpose=True,
                                     start=True, stop=True)
                    nc.scalar.copy(lk_all[:, 128 * k:128 * k + DH], vt_ps[:])
                    nc.vector.tensor_copy(lk_all[:, 128 * k + DH:
                                                 128 * k + DH + 1],
                                          tl_sb[:, k:k + 1])
                lh_all = bpool.tile([128, 128 * KL], BF16, tag="lh_all")
                nc.vector.tensor_copy(lh_all[:], lk_all[:])
                lup = bpool.tile([128, 128 * KL], F32, tag="lup")
                nc.vector.tensor_copy(lup[:], lh_all[:])
                nc.vector.tensor_sub(lup[:], lk_all[:], lup[:])
                ll_all = bpool.tile([128, 128 * KL], BF16, tag="ll_all")
                nc.vector.tensor_copy(ll_all[:], lup[:])

                # ---- matmul1: vf(+theta_links row) = [V;tl]^T-chunks @ D ----
                vf_ps = []
                with tc.tile_pool(name="psV", bufs=1, space="PSUM") as psV:
                    for n in range(NPS):
                        w = min(512, PSHARD - 512 * n)
                        vf_ps.append(psV.tile([128, w], F32, tag=f"vf{n}",
                                              name=f"vf{n}"))
                    for k in range(KL):
                        dk_t = spool.tile([128, PSHARD], BF16, tag="dk")
                        nc.sync.dma_start(dk_t[:], p_d.ap()[k])
                        for half, lt in ((0, lh_all), (1, ll_all)):
                            for n in range(NPS):
                                w = min(512, PSHARD - 512 * n)
                                nc.tensor.matmul(
                                    vf_ps[n][:],
                                    lt[:, 128 * k:128 * (k + 1)],
                                    dk_t[:, 512 * n:512 * n + w],
                                    start=(k == 0 and half == 0),
                                    stop=(k == KL - 1 and half == 1))

                    ysb = bpool.tile([97, PSHARD], F32, tag="ysb")
                    for n in range(NPS):
                        w = min(512, PSHARD - 512 * n)
                        nc.scalar.copy(ysb[:, 512 * n:512 * n + w],
                                       vf_ps[n][0:97, :])

                # ---- transpose vf chunks, exp -> evT (f32 + bf16 copies) ----
                evt = bpool.tile([128, DH * NCH], F32, tag="evt")
                evb = bpool.tile([128, DH * NCH], BF16, tag="evb")
                for c in range(NCH):
                    yt_ps = psA.tile([128, 97], F32, tag="tr")
                    nc.tensor.matmul(yt_ps[:], ysb[:, 128 * c:128 * (c + 1)],
                                     ident[:97, :97], is_transpose=True,
                                     start=True, stop=True)
                    cvec = spool.tile([128, 1], F32, tag="cvec")
                    nc.scalar.copy(cvec[:], yt_ps[:, DH:DH + 1])
                    nc.scalar.activation(evt[:, DH * c:DH * (c + 1)],
                                         yt_ps[:, 0:DH], AF.Exp, bias=cvec[:])
                    nc.vector.tensor_copy(evb[:, DH * c:DH * (c + 1)],
                                          evt[:, DH * c:DH * (c + 1)])

                # ---- segment sums over chunk-local od slots (bf16) ----
                tall = bpool.tile([SLOT, DH * NCH], BF16, tag="tall")
                tlast = bpool.tile([1, DH * NCH], BF16, tag="tlast")
                with tc.tile_pool(name="psS", bufs=2, space="PSUM") as psS:
                    for c in range(NCH):
                        ts_ps = psS.tile([SLOT, DH], F32, tag="seg")
                        nc.tensor.matmul(ts_ps[:],
                                         s_all[:, SLOT * c:SLOT * (c + 1)],
                                         evb[:, DH * c:DH * (c + 1)],
                                         start=True, stop=True)
                        nc.vector.tensor_copy(tall[:, DH * c:DH * (c + 1)],
                                              ts_ps[:])
                        tl_ps = psS.tile([1, DH], F32, tag="segl", bufs=1)
                        nc.tensor.matmul(tl_ps[:], sl_all[:, c:c + 1],
                                         evb[:, DH * c:DH * (c + 1)],
                                         start=True, stop=True)
                        nc.scalar.copy(tlast[:, DH * c:DH * (c + 1)],
                                       tl_ps[:])

                # ---- gather denominators per path; f^T = ev*q/denom ----
                ft_bf = bpool.tile([128, 128 * NCH], BF16, tag="ftb")
                nc.vector.memset(ft_bf[:], 0.0)
                with tc.tile_pool(name="psG", bufs=3, space="PSUM") as psG:
                    for c in range(NCH):
                        g_ps = psG.tile([128, DH], F32, tag="g")
                        cn = (c + 1) % NCH
                        cp = (c - 1) % NCH
                        nc.tensor.matmul(g_ps[:],
                                         stm_all[:, 128 * c:128 * (c + 1)],
                                         tall[:, DH * c:DH * (c + 1)],
                                         start=True, stop=False)
                        nc.tensor.matmul(g_ps[:],
                                         stf_all[:, 128 * c:128 * (c + 1)],
                                         tall[0:1, DH * cn:DH * cn + DH],
                                         start=False, stop=False)
                        nc.tensor.matmul(g_ps[:],
                                         stb_all[:, 128 * c:128 * (c + 1)],
                                         tlast[0:1, DH * cp:DH * cp + DH],
                                         start=False, stop=True)
                        rec = spool.tile([128, DH], F32, tag="rec")
                        nc.vector.tensor_scalar_max(rec[:], g_ps[:], 1e-30)
                        nc.vector.reciprocal(rec[:], rec[:])
                        tmp = spool.tile([128, DH], F32, tag="tmp")
                        nc.vector.scalar_tensor_tensor(
                            tmp[:], evt[:, DH * c:DH * (c + 1)],
                            qsq[:, c:c + 1], rec[:], ALU.mult, ALU.mult)
                        nc.vector.tensor_copy(
                            ft_bf[:, 128 * c:128 * c + DH], tmp[:])

                # ---- matmul2: x_partial = f^T-chunks @ DT ----
                ar_in = dpool.tile([DH, L_PAD], F32, tag="arin")
                ar_out = dpool.tile([DHS, L_PAD], F32, tag="arout")
                with tc.tile_pool(name="psX", bufs=1, space="PSUM") as psX:
                    x_ps = [psX.tile([128, 512], F32, tag=f"x{n}", name=f"x{n}")
                            for n in range(L_PAD // 512)]
                    for c in range(NCH):
                        dt_t = dtpool.tile([128, L_PAD], BF16, tag="dt")
                        nc.sync.dma_start(dt_t[:], p_dt.ap()[c])
                        for n in range(L_PAD // 512):
                            nc.tensor.matmul(
                                x_ps[n][:], ft_bf[:, 128 * c:128 * (c + 1)],
                                dt_t[:, 512 * n:512 * (n + 1)],
                                start=(c == 0), stop=(c == NCH - 1))
                    xp = bpool.tile([DH, L_PAD], F32, tag="xp")
                    for n in range(L_PAD // 512):
                        nc.scalar.copy(xp[:, 512 * n:512 * (n + 1)],
                                       x_ps[n][0:DH, :])
                        nc.sync.dma_start(ar_in[:, 512 * n:512 * (n + 1)],
                                          xp[:, 512 * n:512 * (n + 1)])

                # ---- ReduceScatter partial link flows; each core keeps its
                # own DHS day-hour rows ----
                nc.gpsimd.collective_compute(
                    "ReduceScatter", ALU.add,
                    replica_groups=[list(range(NCORES))],
                    ins=[ar_in.opt()], outs=[ar_out.opt()])

                # ---- BPR epilogue in the folded (96, 256) layout ----
                xg = bpool.tile([DH, 256], F32, tag="xg")
                nc.sync.dma_start(
                    xg[:], ar_out.rearrange("d (a l) -> (d a) l", a=FB))
                t0 = bpool.tile([DH, 256], F32, tag="t0")
                nc.vector.tensor_mul(t0[:], xg[:], ib[:])
                nc.vector.tensor_scalar_max(t0[:], t0[:], 1e-35)
                t1 = bpool.tile([DH, 256], F32, tag="t1")
                nc.scalar.activation(t1[:], t0[:], AF.Ln)
                nc.vector.tensor_mul(t1[:], t1[:], bb2[:])
                t2 = bpool.tile([DH, 256], F32, tag="t2")
                nc.scalar.activation(t2[:], t1[:], AF.Exp)
                nc.vector.tensor_mul(t2[:], t2[:], atf[:])
                o_t = bpool.tile([DH, 256], F32, tag="o")
                nc.vector.tensor_add(o_t[:], t2[:], ttf[:])
                nc.sync.dma_start(p_out.ap(), o_t[:])

    nc.compile()
    return nc


_CACHE = {}
LAST_RESULT = None


def _get_program(PSHARD, SLOT):
    key = (PSHARD, SLOT)
    if key not in _CACHE:
        _CACHE[key] = _build_program(PSHARD, SLOT)
    return _CACHE[key]


def _fold96(v_lpad):
    """(L_PAD,) per-link vector -> (96, 256) folded layout (row 8*d + a holds
    link block [256a, 256(a+1)) for every local day-hour d)."""
    return np.ascontiguousarray(
        np.tile(v_lpad.reshape(FB, 256), (DHS, 1)).astype(np.float32))


def kernel(X, theta_raw, theta_links, q_sqrt, log_alpha, beta_raw, k, D,
           od_of_path, n_ods):
    X = np.asarray(X, np.float32)
    D = np.asarray(D, np.float32)
    od = np.asarray(od_of_path, np.int32)
    assert X.shape == (ND, NH, NL, NF + 1) and D.shape == (NL, NP)
    assert int(n_ods) == NOD

    od_per_core = (NOD + NCORES - 1) // NCORES
    bounds = np.searchsorted(od, np.arange(0, NOD + 1, od_per_core)[:NCORES + 1])
    bounds[0], bounds[-1] = 0, NP
    cnts = np.diff(bounds)
    PSHARD = int(np.ceil(cnts.max() / 128) * 128)
    NCH = PSHARD // 128

    max_span = 1
    for i in range(NCORES):
        odl = od[bounds[i]:bounds[i + 1]]
        for c in range(0, len(odl), 128):
            ch = odl[c:c + 128]
            if len(ch):
                max_span = max(max_span, int(ch[-1] - ch[0]) + 1)
    W = int(np.ceil(max_span / 32) * 32)
    SLOT = W

    nc = _get_program(PSHARD, SLOT)

    # ---- host-side shard construction (index bookkeeping + relayout only) --
    Xf = X.reshape(DH, NL, NF + 1)
    ttf_full = np.zeros((DH, L_PAD), np.float32)
    ttf_full[:, :NL] = Xf[:, :, 0]
    ft_h = np.zeros((DH, NF, L_PAD), np.float32)
    for f in range(NF):
        ft_h[:, f, :NL] = Xf[:, :, f + 1]
    ft_h = np.ascontiguousarray(ft_h.reshape(DH, NF * L_PAD))

    def padded_vec(v, fill=0.0):
        o = np.full(L_PAD, fill, np.float32)
        o[:NL] = v
        return o

    tl_h = np.ascontiguousarray(
        padded_vec(np.asarray(theta_links, np.float32)).reshape(KL, 128).T)
    kb_h = _fold96(padded_vec(np.asarray(k, np.float32), fill=1.0))
    bb_h = _fold96(padded_vec(np.asarray(beta_raw, np.float32)))
    lab_h = _fold96(padded_vec(np.asarray(log_alpha, np.float32)))
    th_h = np.asarray(theta_raw, np.float32).reshape(1, NF)
    qsr = np.asarray(q_sqrt, np.float32)
    id_h = np.eye(128, dtype=np.float32)

    in_maps = []
    for i in range(NCORES):
        lo, hi = bounds[i], bounds[i + 1]
        cnt = hi - lo
        odl = od[lo:hi]

        Dsh = np.zeros((L_PAD, PSHARD), np.float32)
        Dsh[:NL, :cnt] = D[:, lo:hi]
        dk_h = Dsh.astype(ml_dtypes.bfloat16).reshape(KL, 128, PSHARD)
        dt_h = np.ascontiguousarray(Dsh.T).astype(
            ml_dtypes.bfloat16).reshape(NCH, 128, L_PAD)

        s_h = np.zeros((128, NCH, SLOT), ml_dtypes.bfloat16)
        sl_h = np.zeros((128, NCH), ml_dtypes.bfloat16)
        stm_h = np.zeros((SLOT, NCH, 128), ml_dtypes.bfloat16)
        stf_h = np.zeros((1, NCH, 128), ml_dtypes.bfloat16)
        stb_h = np.zeros((1, NCH, 128), ml_dtypes.bfloat16)
        qs_h = np.zeros(PSHARD, np.float32)
        qs_h[:cnt] = qsr[odl]
        qs_h = np.ascontiguousarray(qs_h.reshape(NCH, 128).T)

        firsts, lasts = {}, {}
        for c in range(NCH):
            ch = odl[128 * c:128 * (c + 1)]
            if len(ch):
                firsts[c], lasts[c] = int(ch[0]), int(ch[-1])
        for c in range(NCH):
            ch = odl[128 * c:128 * (c + 1)]
            if not len(ch):
                continue
            f0, l0 = firsts[c], lasts[c]
            asc = ch - f0
            rows = np.arange(len(ch))
            s_h[rows, c, asc] = 1.0
            sl_h[rows[ch == l0], c] = 1.0
            stm_h[asc, c, rows] = 1.0
            if c + 1 in firsts and firsts[c + 1] == l0:
                stf_h[0, c, rows[ch == l0]] = 1.0
            if c - 1 in lasts and lasts[c - 1] == f0:
                stb_h[0, c, rows[ch == f0]] = 1.0

        in_maps.append(dict(
            ft=ft_h, dk=dk_h, dtk=dt_h,
            seg=np.ascontiguousarray(s_h.reshape(128, NCH * SLOT)),
            segl=np.ascontiguousarray(sl_h),
            stm=np.ascontiguousarray(stm_h.reshape(SLOT, NCH * 128)),
            stf=np.ascontiguousarray(stf_h.reshape(1, NCH * 128)),
            stb=np.ascontiguousarray(stb_h.reshape(1, NCH * 128)),
            qsp=qs_h, th=th_h, tl=tl_h, idn=id_h,
            kb96=kb_h, bb96=bb_h, lab96=lab_h,
            ttf96=np.ascontiguousarray(
                ttf_full[DHS * i:DHS * (i + 1)].reshape(DH, 256))))

    trace = os.environ.get("BASS_KERNEL_TRACE", "0") == "1"
    res = run_bass_kernel_spmd(nc, in_maps, core_ids=list(range(NCORES)),
                               trace=trace)
    global LAST_RESULT
    LAST_RESULT = res
    # core i's (96, 256) folded output = (12, 2048) day-hour rows
    # [DHS*i, DHS*(i+1))
    parts = [r["out"].reshape(DHS, L_PAD) for r in res.results]
    out = np.concatenate(parts, axis=0)[:, :NL]
    return np.ascontiguousarray(out).reshape(ND, NH, NL).astype(np.float32)
